# revision 1
# baseline (speedup 1.0000x reference)
"""Trainium2 Bass kernel for nn_MlroleNode_64716567216639 (GAT message passing).

Math note: the reference model computes a dense NxN GATv2 attention but only
row 0 of the output (gat_out[0]) feeds the final MLP, so this kernel computes
just that row: e[j,h] = leaky(g_l[j] + g_r[0]) . w_attn, softmax over the 1024
source nodes, then a weighted sum of g_r values, followed by the 3-layer
type-define MLP over the 1023 ambiguous nodes.

Layout: features on partitions, nodes on the free axis (everything transposed
on host). The GAT row-0 computation is replicated on all 8 cores; the final
MLP is sharded 128 nodes per core.
"""
import numpy as np

H = 64
N_AMB = 1023
N = 1024
HEADS = 4
HID = 64
RT = 4
APT = 3
SLOPE = 0.2
NCORES = 8
SHARD = 128  # MLP nodes per core (8*128 = 1024 = N_AMB padded by 1)

_compiled = None  # (nc, static_in_maps_builder)


def _build():
    import concourse.tile as tile
    from concourse import bacc, mybir

    dt = mybir.dt.float32
    AF = mybir.ActivationFunctionType
    ALU = mybir.AluOpType
    AX = mybir.AxisListType

    nc = bacc.Bacc("TRN2", target_bir_lowering=False, debug=False,
                   enable_asserts=False, num_devices=NCORES)

    def din(name, shape):
        return nc.dram_tensor(name, shape, dt, kind="ExternalInput").ap()

    ambT_d = din("ambT", [H, N_AMB])
    hidc_d = din("hidc", [H, 1])
    ta_d = din("ta", [H, RT * APT])
    WselfT_d = din("WselfT", [H, H])
    WmLT_d = din("WmLT", [H, H])
    WmRT_d = din("WmRT", [H, H])
    WtT_d = din("WtT", [H, RT * H])
    btT_d = din("btT", [H, RT])
    bsc_d = din("bsc", [H, 1])
    bmc_d = din("bmc", [H, 1])
    WlT0_d = din("WlT0", [H, 128])
    WlT1_d = din("WlT1", [H, 128])
    WrT_d = din("WrT", [H, HEADS * HID])
    Wexp_d = din("Wexp", [128, 128])
    fold_d = din("fold", [128, H])
    Wd0a_aug_d = din("Wd0a_aug", [H + 1, 64])
    Wd0bT_d = din("Wd0bT", [H, 64])
    Wd1_aug_d = din("Wd1_aug", [65, 128])
    Wd2T_d = din("Wd2T", [128, RT])
    bd2c_d = din("bd2c", [RT, 1])
    mlp_d = din("mlp_cols", [H, SHARD])
    outT_d = nc.dram_tensor("outT", [RT, SHARD], dt, kind="ExternalOutput").ap()

    with tile.TileContext(nc) as tc:
        with tc.tile_pool(name="wp", bufs=1) as wp, \
             tc.tile_pool(name="sb", bufs=1) as sb, \
             tc.tile_pool(name="ps", bufs=1, space="PSUM") as ps:

            # ---- load inputs to SBUF ----
            def load(dram_ap, shape, tag):
                t = wp.tile(shape, dt, tag=tag)
                nc.sync.dma_start(t[:], dram_ap[:])
                return t

            ta_sb = load(ta_d, [H, RT * APT], "ta")
            WselfT = load(WselfT_d, [H, H], "WselfT")
            WmLT = load(WmLT_d, [H, H], "WmLT")
            WmRT = load(WmRT_d, [H, H], "WmRT")
            WtT = load(WtT_d, [H, RT * H], "WtT")
            btT = load(btT_d, [H, RT], "btT")
            bsc = load(bsc_d, [H, 1], "bsc")
            bmc = load(bmc_d, [H, 1], "bmc")
            hidc = load(hidc_d, [H, 1], "hidc")
            WlT = [load(WlT0_d, [H, 128], "WlT0"), load(WlT1_d, [H, 128], "WlT1")]
            WrT = load(WrT_d, [H, HEADS * HID], "WrT")
            Wexp = load(Wexp_d, [128, 128], "Wexp")
            fold = load(fold_d, [128, H], "fold")
            Wd0a_aug = load(Wd0a_aug_d, [H + 1, 64], "Wd0a")
            Wd0bT = load(Wd0bT_d, [H, 64], "Wd0b")
            Wd1_aug = load(Wd1_aug_d, [65, 128], "Wd1")
            Wd2T = load(Wd2T_d, [128, RT], "Wd2")
            bd2c = load(bd2c_d, [RT, 1], "bd2c")

            hT = wp.tile([H, N], dt, tag="hT")
            nc.sync.dma_start(hT[:, 1:N], ambT_d[:])
            mlp_aug = wp.tile([H + 1, SHARD], dt, tag="mlpa")
            nc.sync.dma_start(mlp_aug[0:H, :], mlp_d[:])
            nc.vector.memset(mlp_aug[H:H + 1, :], 1.0)
            # preload ACT tables (Exp/Sigmoid) off the critical softmax path
            warm = wp.tile([1, 4], dt, tag="warm")
            nc.vector.memset(warm[:], 0.0)
            warm_act = wp.tile([1, 4], dt, tag="warmact")
            nc.scalar.activation(warm_act[0:1, 0:1], warm[0:1, 0:1], AF.Exp)

            def leaky(out_ap, in_ap):
                # in_ap must be SBUF (stt can read at most one PSUM input)
                nc.vector.scalar_tensor_tensor(out=out_ap, in0=in_ap, scalar=SLOPE,
                                               in1=in_ap, op0=ALU.mult, op1=ALU.max)

            def leaky_psum(out_ap, psum_ap, scratch_ap):
                # leaky(x) = max(0.2*x, x) with x in PSUM: two DVE ops
                nc.vector.tensor_scalar_mul(scratch_ap, psum_ap, SLOPE)
                nc.vector.tensor_tensor(out_ap, scratch_ap, psum_ap, op=ALU.max)

            # ---- prologue: role-type routing + merge chain -> h1 [64,1] ----
            tsum = sb.tile([H, RT], dt, tag="tsum")
            nc.vector.reduce_sum(tsum[:], ta_sb[:].rearrange("p (t a) -> p t a", a=APT),
                                 axis=AX.X)
            tmean = sb.tile([H, RT], dt, tag="tmean")
            nc.vector.tensor_scalar_mul(tmean[:], tsum[:], 1.0 / APT)
            tmp_ps = ps.tile([H, RT], dt, tag="sp", bufs=1)
            for t in range(RT):
                nc.tensor.matmul(tmp_ps[:, t:t + 1], WtT[:, H * t:H * (t + 1)],
                                 tmean[:, t:t + 1], start=True, stop=True)
            tmpc = sb.tile([H, RT], dt, tag="tmpc")
            nc.vector.tensor_add(tmpc[:], tmp_ps[:], btT[:])
            C_ps = ps.tile([H, RT], dt, tag="sp", bufs=1)
            nc.tensor.matmul(C_ps[:], WmRT[:], tmpc[:], start=True, stop=True)
            C_sb = sb.tile([H, RT], dt, tag="C")
            nc.vector.tensor_scalar_add(C_sb[:], C_ps[:], bmc[:])

            h1_ps = ps.tile([H, 1], dt, tag="sp", bufs=1)
            nc.tensor.matmul(h1_ps[:], WselfT[:], hidc[:], start=True, stop=True)
            h1 = sb.tile([H, 1], dt, tag="h1", bufs=2)
            nc.vector.tensor_scalar_add(h1[:], h1_ps[:], bsc[:])
            for t in range(RT):
                hp = ps.tile([H, 1], dt, tag="sp", bufs=1)
                nc.tensor.matmul(hp[:], WmLT[:], h1[:], start=True, stop=True)
                u = sb.tile([H, 1], dt, tag="u", bufs=2)
                nc.vector.tensor_scalar_add(u[:], hp[:], C_sb[:, t:t + 1])
                h1n = sb.tile([H, 1], dt, tag="h1", bufs=2)
                leaky(h1n[:], u[:])
                h1 = h1n
            nc.vector.tensor_copy(hT[:, 0:1], h1[:])

            # ---- GAT row 0, two head-pair blocks ----
            h2_ps = ps.tile([H, 1], dt, tag="h2ps", bufs=1)
            for b in range(2):
                # g_r0 column for this head-pair block (attention query side)
                gr0_ps = ps.tile([128, 1], dt, tag="sp", bufs=1)
                nc.tensor.matmul(gr0_ps[:], WrT[:, 128 * b:128 * b + 128], h1[:],
                                 start=True, stop=True)
                gr0c = sb.tile([128, 1], dt, tag="gr0", bufs=2)
                nc.vector.tensor_copy(gr0c[:], gr0_ps[:])
                gl_ps = ps.tile([128, N], dt, tag="gle", bufs=2)
                for c in (0, 512):
                    nc.tensor.matmul(gl_ps[:, c:c + 512], WlT[b][:], hT[:, c:c + 512],
                                     start=True, stop=True)
                t_sb = sb.tile([128, N], dt, tag="t", bufs=2)
                u_sb = sb.tile([128, N], dt, tag="scr", bufs=2)
                nc.scalar.activation(u_sb[:], gl_ps[:], AF.Identity, bias=gr0c[:])
                leaky(t_sb[:], u_sb[:])
                gr_ps = ps.tile([128, N], dt, tag="gr", bufs=1)
                for c in (0, 512):
                    nc.tensor.matmul(gr_ps[:, c:c + 512],
                                     WrT[:, 128 * b:128 * b + 128],
                                     hT[:, c:c + 512], start=True, stop=True)
                e_ps = ps.tile([128, N], dt, tag="gle", bufs=2)
                for c in (0, 512):
                    nc.tensor.matmul(e_ps[:, c:c + 512], Wexp[:], t_sb[:, c:c + 512],
                                     start=True, stop=True)
                # softmax over the 1024 source nodes (per head, replicated x64).
                # logits are O(5) so no max subtraction is needed in fp32.
                pexp = sb.tile([128, N], dt, tag="pexp", bufs=2)
                ssum = sb.tile([128, 1], dt, tag="s", bufs=4)
                nc.scalar.activation(pexp[:], e_ps[:], AF.Exp, bias=0.0,
                                     accum_out=ssum[:])
                # weighted value sum over source nodes (fused mul + row-sum)
                scr = sb.tile([128, N], dt, tag="scr", bufs=2)
                att_u = sb.tile([128, 1], dt, tag="acc", bufs=4)
                nc.vector.scalar_tensor_tensor(
                    out=scr[:], in0=pexp[:], scalar=1.0, in1=gr_ps[:],
                    op0=ALU.mult, op1=ALU.mult, accum_out=att_u[:])
                rs = sb.tile([128, 1], dt, tag="s", bufs=4)
                nc.vector.reciprocal(rs[:], ssum[:])
                att_n = sb.tile([128, 1], dt, tag="acc", bufs=4)
                nc.vector.tensor_mul(att_n[:], att_u[:], rs[:])
                # fold heads: h2 += 0.25 * sum over the 2 heads in this block
                nc.tensor.matmul(h2_ps[:], fold[:], att_n[:], start=(b == 0),
                                 stop=(b == 1))

            h2 = sb.tile([H, 1], dt, tag="h2")
            nc.vector.tensor_copy(h2[:], h2_ps[:])

            # ---- final MLP on this core's 128-node shard ----
            c0_ps = ps.tile([H, 1], dt, tag="sp", bufs=1)
            nc.tensor.matmul(c0_ps[:], Wd0bT[:], h2[:], start=True, stop=True)
            c0col = sb.tile([H, 1], dt, tag="c0")
            nc.vector.tensor_copy(c0col[:], c0_ps[:])
            y0_ps = ps.tile([64, SHARD], dt, tag="sp", bufs=1)
            nc.tensor.matmul(y0_ps[:], Wd0a_aug[:], mlp_aug[:], start=True, stop=True)
            y0_aug = sb.tile([65, SHARD], dt, tag="y0")
            nc.vector.memset(y0_aug[64:65, :], 1.0)
            y0u = sb.tile([64, SHARD], dt, tag="yscr", bufs=2)
            nc.scalar.activation(y0u[:], y0_ps[:], AF.Identity, bias=c0col[:])
            leaky(y0_aug[0:64, :], y0u[:])
            y1_ps = ps.tile([128, SHARD], dt, tag="sp", bufs=1)
            nc.tensor.matmul(y1_ps[:], Wd1_aug[:], y0_aug[:], start=True, stop=True)
            y1 = sb.tile([128, SHARD], dt, tag="y1")
            y1scr = sb.tile([128, SHARD], dt, tag="yscr", bufs=2)
            leaky_psum(y1[:], y1_ps[:], y1scr[:])
            o_ps = ps.tile([RT, SHARD], dt, tag="sp", bufs=1)
            nc.tensor.matmul(o_ps[:], Wd2T[:], y1[:], start=True, stop=True)
            # sigmoid(z) = 1/(1+exp(-z)) using the already-loaded Exp table
            # (avoids a 1.3us Sigmoid ACT-table load on the critical path)
            o_e = sb.tile([RT, SHARD], dt, tag="oe")
            nc.scalar.activation(o_e[:], o_ps[:], AF.Exp, bias=bd2c[:], scale=-1.0)
            o_1p = sb.tile([RT, SHARD], dt, tag="o1p")
            nc.vector.tensor_scalar_add(o_1p[:], o_e[:], 1.0)
            o_sb = sb.tile([RT, SHARD], dt, tag="o")
            nc.vector.reciprocal(o_sb[:], o_1p[:])
            nc.sync.dma_start(outT_d[:], o_sb[:])

    nc.compile()
    return nc


def _prep_inputs(inputs):
    f32 = np.float32

    def c(a):
        return np.ascontiguousarray(a, dtype=f32)

    hidden = np.asarray(inputs["hidden"], f32)
    ambiguous = np.asarray(inputs["ambiguous"], f32)
    type_agents = np.asarray(inputs["type_agents"], f32)
    W_self = np.asarray(inputs["W_self"], f32)
    b_self = np.asarray(inputs["b_self"], f32)
    W_merge = np.asarray(inputs["W_merge"], f32)
    b_merge = np.asarray(inputs["b_merge"], f32)
    W_trans = np.asarray(inputs["W_trans"], f32)
    b_trans = np.asarray(inputs["b_trans"], f32)
    W_l = np.asarray(inputs["W_l"], f32)
    W_r = np.asarray(inputs["W_r"], f32)
    w_attn = np.asarray(inputs["w_attn"], f32)
    Wd0 = np.asarray(inputs["Wd0"], f32)
    bd0 = np.asarray(inputs["bd0"], f32)
    Wd1 = np.asarray(inputs["Wd1"], f32)
    bd1 = np.asarray(inputs["bd1"], f32)
    Wd2 = np.asarray(inputs["Wd2"], f32)
    bd2 = np.asarray(inputs["bd2"], f32)

    ambT = c(ambiguous.T)                                   # [64, 1023]
    WlT_full = c(W_l.T)                                     # [64, 256]
    Wexp = np.zeros((128, 128), f32)
    for hh in range(2):
        Wexp[hh * 64:(hh + 1) * 64, hh * 64:(hh + 1) * 64] = w_attn[:, None]
    fold = np.zeros((128, 64), f32)
    fold[np.arange(128), np.arange(128) % 64] = 0.25

    shared = {
        "ambT": ambT,
        "hidc": c(hidden.reshape(H, 1)),
        "ta": c(type_agents.reshape(RT * APT, H).T),
        "WselfT": c(W_self.T),
        "WmLT": c(W_merge[:, :H].T),
        "WmRT": c(W_merge[:, H:].T),
        "WtT": c(np.concatenate([W_trans[t].T for t in range(RT)], axis=1)),
        "btT": c(b_trans.T),
        "bsc": c(b_self.reshape(H, 1)),
        "bmc": c(b_merge.reshape(H, 1)),
        "WlT0": c(WlT_full[:, :128]),
        "WlT1": c(WlT_full[:, 128:]),
        "WrT": c(W_r.T),
        "Wexp": Wexp,
        "fold": fold,
        "Wd0a_aug": c(np.vstack([Wd0[:, :H].T, bd0[None, :]])),
        "Wd0bT": c(Wd0[:, H:].T),
        "Wd1_aug": c(np.vstack([Wd1.T, bd1[None, :]])),
        "Wd2T": c(Wd2.T),
        # negated: used as the bias of Exp(-z) inside the exp-based sigmoid
        "bd2c": c(-bd2.reshape(RT, 1)),
    }
    amb_pad = np.zeros((H, NCORES * SHARD), f32)
    amb_pad[:, :N_AMB] = ambT
    in_maps = []
    for cidx in range(NCORES):
        m = dict(shared)
        m["mlp_cols"] = c(amb_pad[:, cidx * SHARD:(cidx + 1) * SHARD])
        in_maps.append(m)
    return in_maps


def kernel(**inputs) -> np.ndarray:
    global _compiled
    if _compiled is None:
        _compiled = _build()
    nc = _compiled
    from concourse import bass_utils

    in_maps = _prep_inputs(inputs)
    res = bass_utils.run_bass_kernel_spmd(nc, in_maps, core_ids=list(range(NCORES)))
    out = np.empty((N_AMB, RT), np.float32)
    for cidx in range(NCORES):
        lo = cidx * SHARD
        hi = min(lo + SHARD, N_AMB)
        out[lo:hi, :] = res.results[cidx]["outT"][:, :hi - lo].T
    return out



# revision 6
# speedup vs baseline: 1.6774x; 1.6774x over previous
"""Trainium2 Bass kernel for nn_MlroleNode_64716567216639 (GAT message passing).

Math note: the reference computes a dense NxN GATv2 attention but only row 0
of the output (gat_out[0]) feeds the final MLP, so this kernel computes just
that row: e[j,h] = leaky(g_l[j] + g_r[0]) . w_attn over the 1024 source nodes,
softmax, weighted sum of g_r, then the 3-layer type-define MLP over the 1023
ambiguous nodes.

Structure: the 1023 ambiguous source columns are processed independently of
the serial role-routing prologue (which produces node 0's embedding h1); node
0's own attention contribution is added as a [128,1] fixup afterwards. All
matmuls and large elementwise ops run in bf16 (fp32 PSUM accumulation);
biases are folded into augmented weight rows host-side. Inputs arrive in 6
packed DMAs ordered so the prologue and GAT start as early as possible. The
GAT row-0 computation is replicated on all 8 cores; the final MLP is sharded
128 nodes per core.
"""
import numpy as np

H = 64
N_AMB = 1023
HEADS = 4
HID = 64
RT = 4
APT = 3
SLOPE = 0.2
NCORES = 8
SHARD = 128

# column offsets in the packed "pro" tensor [65, 461]
PRO_WSELF = 0          # WselfT_aug [65,64]
PRO_WT = 64            # WtT_aug (4x [65,64], 1/3-scaled, b_trans row)
PRO_WMR = 320          # WmRT_aug [65,64] (b_merge row)
PRO_WML = 384          # WmLT [64,64]
PRO_HID = 448          # hidc_aug [65,1]
PRO_TA = 449           # ta [64,12]
PRO_COLS = 461

# column offsets in the packed "gatw" tensor [64, 1535]
GW_AMB = 0             # ambT [64,1023]
GW_WL = 1023           # WlT [64,256] (2 blocks of 128)
GW_WR = 1279           # WrT [64,256]
GW_COLS = 1535

# column offsets in the packed "mlpw" tensor [128, 260]
MW_G = 0               # G = 0.25*[Wd0b.T; Wd0b.T] [128,64]
MW_WD1 = 64            # Wd1_aug [65,128]
MW_WD0A = 192          # Wd0a_aug [65,64]
MW_WD2 = 256           # Wd2T [128,4]
MW_COLS = 260

_compiled = None


def _build():
    import concourse.tile as tile
    from concourse import bacc, mybir

    f32 = mybir.dt.float32
    bf16 = mybir.dt.bfloat16
    AF = mybir.ActivationFunctionType
    ALU = mybir.AluOpType
    AX = mybir.AxisListType

    nc = bacc.Bacc("TRN2", target_bir_lowering=False, debug=False,
                   enable_asserts=False, num_devices=NCORES)

    pro_d = nc.dram_tensor("pro", [65, PRO_COLS], bf16, kind="ExternalInput").ap()
    gatw_d = nc.dram_tensor("gatw", [64, GW_COLS], bf16, kind="ExternalInput").ap()
    wexp_d = nc.dram_tensor("wexp", [128, 128], bf16, kind="ExternalInput").ap()
    mlpw_d = nc.dram_tensor("mlpw", [128, MW_COLS], bf16, kind="ExternalInput").ap()
    mlpin_d = nc.dram_tensor("mlpin", [65, SHARD], bf16, kind="ExternalInput").ap()
    bd2h_d = nc.dram_tensor("bd2h", [RT, 1], f32, kind="ExternalInput").ap()
    outT_d = nc.dram_tensor("outT", [RT, SHARD], f32, kind="ExternalOutput").ap()

    NA = N_AMB  # 1023
    C0, C1 = 512, N_AMB  # column chunks [0:512), [512:1023)

    with tile.TileContext(nc) as tc:
        with tc.tile_pool(name="wp", bufs=1) as wp, \
             tc.tile_pool(name="sb", bufs=1) as sb, \
             tc.tile_pool(name="ps", bufs=1, space="PSUM") as ps:

            # ---- input DMAs, critical-first; mlp set on gpsimd queue ----
            pro = wp.tile([65, PRO_COLS], bf16, tag="pro")
            nc.sync.dma_start(pro[:], pro_d[:])
            gatw = wp.tile([64, GW_COLS], bf16, tag="gatw")
            nc.sync.dma_start(gatw[:], gatw_d[:])
            wexp = wp.tile([128, 128], bf16, tag="wexp")
            nc.sync.dma_start(wexp[:], wexp_d[:])
            mlpw = wp.tile([128, MW_COLS], bf16, tag="mlpw")
            nc.gpsimd.dma_start(mlpw[:], mlpw_d[:])
            mlpin = wp.tile([65, SHARD], bf16, tag="mlpin")
            nc.gpsimd.dma_start(mlpin[:], mlpin_d[:])
            bd2h = wp.tile([RT, 1], f32, tag="bd2h")
            nc.gpsimd.dma_start(bd2h[:], bd2h_d[:])

            # ---- ACT table warm (Exp/Tanh/Identity share one table set) ----
            warm = sb.tile([1, 1], f32, tag="warm")
            nc.vector.memset(warm[:], 0.0)
            warm_o = sb.tile([1, 1], f32, tag="warmo")
            nc.scalar.activation(warm_o[:], warm[:], AF.Exp)

            # ---- ones rows for augmented operands ----
            tsum = sb.tile([65, RT], bf16, tag="tsum")
            nc.vector.memset(tsum[64:65, :], 1.0)
            tmpc = sb.tile([65, RT], bf16, tag="tmpc")
            nc.vector.memset(tmpc[64:65, :], 1.0)
            y0a = sb.tile([65, SHARD], bf16, tag="y0a")
            nc.vector.memset(y0a[64:65, :], 1.0)

            def leaky(out_ap, in_ap):
                nc.vector.scalar_tensor_tensor(out=out_ap, in0=in_ap, scalar=SLOPE,
                                               in1=in_ap, op0=ALU.mult, op1=ALU.max)

            # ---- prologue: role-type routing (C path) ----
            with nc.allow_low_precision(reason="3-way sum of bf16 agent vectors"):
                nc.vector.reduce_sum(
                    tsum[0:64, :],
                    pro[0:64, PRO_TA:PRO_TA + RT * APT].rearrange("p (t a) -> p t a", a=APT),
                    axis=AX.X)
            tmp_ps = ps.tile([H, RT], f32, tag="spA", padded_shape=[128, 512], name="tmp_ps")
            for t in range(RT):
                nc.tensor.matmul(tmp_ps[:, t:t + 1],
                                 pro[0:65, PRO_WT + H * t:PRO_WT + H * (t + 1)],
                                 tsum[:, t:t + 1], start=True, stop=True)
            nc.vector.tensor_copy(tmpc[0:64, :], tmp_ps[:])
            C_ps = ps.tile([H, RT], f32, tag="spA", padded_shape=[128, 512], name="C_ps")
            nc.tensor.matmul(C_ps[:], pro[0:65, PRO_WMR:PRO_WMR + H], tmpc[:],
                             start=True, stop=True)
            C_sb = sb.tile([H, RT], f32, tag="Csb")
            nc.vector.tensor_copy(C_sb[:], C_ps[:])

            # ---- prologue: h1 merge chain (node 0 embedding) ----
            h1_ps = ps.tile([H, 1], f32, tag="spA", padded_shape=[128, 512], name="h1_ps")
            nc.tensor.matmul(h1_ps[:], pro[0:65, PRO_WSELF:PRO_WSELF + H],
                             pro[0:65, PRO_HID:PRO_HID + 1], start=True, stop=True)
            h1 = sb.tile([H, 1], bf16, tag="h1", bufs=2)
            nc.vector.tensor_copy(h1[:], h1_ps[:])

            gl_ps = ps.tile([128, NA], f32, tag="gl", bufs=2)
            gr_ps = ps.tile([128, NA], f32, tag="gr", bufs=1)
            gl_ps2 = None
            gr_ps2 = None
            big_mms = []  # (out_chunk, lhsT, rhs_chunk) interleaved into chain
            for b in range(2):
                glp = gl_ps if b == 0 else ps.tile([128, NA], f32, tag="gl", bufs=2)
                grp = gr_ps if b == 0 else ps.tile([128, NA], f32, tag="gr", bufs=1)
                if b == 1:
                    gl_ps2, gr_ps2 = glp, grp
                for (lo, hi), wofs, dst in (
                        ((0, C0), GW_WL, glp), ((C0, C1), GW_WL, glp),
                        ((0, C0), GW_WR, grp), ((C0, C1), GW_WR, grp)):
                    big_mms.append((dst, gatw[0:64, wofs + 128 * b:wofs + 128 * (b + 1)],
                                    (lo, hi)))
            gl_b = [gl_ps, gl_ps2]
            gr_b = [gr_ps, gr_ps2]

            # serial chain with block-0 big matmuls interleaved on the PE
            mm_iter = iter(big_mms)
            chain_ps = ps.tile([H, 1], f32, tag="spA", padded_shape=[128, 512], name="chain_ps")
            for t in range(RT):
                nc.tensor.matmul(chain_ps[:], pro[0:64, PRO_WML:PRO_WML + H], h1[:],
                                 start=True, stop=True)
                u = sb.tile([H, 1], bf16, tag="u", bufs=2)
                nc.vector.tensor_scalar_add(u[:], chain_ps[:], C_sb[:, t:t + 1])
                h1n = sb.tile([H, 1], bf16, tag="h1", bufs=2)
                leaky(h1n[:], u[:])
                h1 = h1n
                dst, lhsT, (lo, hi) = next(mm_iter)
                nc.tensor.matmul(dst[:, lo:hi], lhsT, gatw[0:64, lo:hi],
                                 start=True, stop=True)

            # gr0 (attention query) + j=0 g_l column, then rest of big matmuls
            gr0_ps = ps.tile([128, 2], f32, tag="spA", padded_shape=[128, 512], name="gr0_ps")
            for b in range(2):
                nc.tensor.matmul(gr0_ps[:, b:b + 1],
                                 gatw[0:64, GW_WR + 128 * b:GW_WR + 128 * (b + 1)],
                                 h1[:], start=True, stop=True)
            gr0c = sb.tile([128, 2], f32, tag="gr0c")
            nc.scalar.activation(gr0c[:, 0:1], gr0_ps[:, 0:1], AF.Identity)
            nc.scalar.activation(gr0c[:, 1:2], gr0_ps[:, 1:2], AF.Identity)
            gl0_ps = ps.tile([128, 2], f32, tag="spA", padded_shape=[128, 512], name="gl0_ps")
            for b in range(2):
                nc.tensor.matmul(gl0_ps[:, b:b + 1],
                                 gatw[0:64, GW_WL + 128 * b:GW_WL + 128 * (b + 1)],
                                 h1[:], start=True, stop=True)
            for dst, lhsT, (lo, hi) in mm_iter:
                nc.tensor.matmul(dst[:, lo:hi], lhsT, gatw[0:64, lo:hi],
                                 start=True, stop=True)

            # ---- GAT pipeline over the 1023 source columns ----
            ssum = sb.tile([128, 2], f32, tag="ssum")
            att_u = sb.tile([128, 2], f32, tag="attu")
            p0c = sb.tile([128, 2], f32, tag="p0c")
            u0 = sb.tile([128, 2], bf16, tag="u0")
            t_sb = [None, None]

            # block 0: bias-add (ACT) then leaky (DVE, 2x bf16)
            tu0 = sb.tile([128, NA], bf16, tag="tu", bufs=2)
            nc.scalar.activation(tu0[:], gl_b[0][:], AF.Identity, bias=gr0c[:, 0:1])
            t_sb[0] = sb.tile([128, NA], bf16, tag="tl", bufs=2, name="tsb0")
            leaky(t_sb[0][:], tu0[:])
            # j=0 fixup pre-activations for both blocks
            u0u = sb.tile([128, 2], bf16, tag="u0u")
            nc.vector.tensor_tensor(u0u[:], gl0_ps[:], gr0c[:], op=ALU.add)
            leaky(u0[:], u0u[:])
            # block 1 bias-add + leaky
            tu1 = sb.tile([128, NA], bf16, tag="tu", bufs=2)
            nc.scalar.activation(tu1[:], gl_b[1][:], AF.Identity, bias=gr0c[:, 1:2])
            t_sb[1] = sb.tile([128, NA], bf16, tag="tl", bufs=2, name="tsb1")
            leaky(t_sb[1][:], tu1[:])

            # e = Wexp @ t (per-head logits replicated across the head's rows)
            e_b = [None, None]
            e_b[0] = ps.tile([128, NA], f32, tag="gl", bufs=2, name="eb0")
            for lo, hi in ((0, C0), (C0, C1)):
                nc.tensor.matmul(e_b[0][:, lo:hi], wexp[:], t_sb[0][:, lo:hi],
                                 start=True, stop=True)
            pexp0 = sb.tile([128, NA], bf16, tag="pex", bufs=2)
            nc.scalar.activation(pexp0[:], e_b[0][:], AF.Exp, bias=0.0,
                                 accum_out=ssum[:, 0:1])
            e0f_ps = ps.tile([128, 2], f32, tag="spA", padded_shape=[128, 512], name="e0f_ps")
            for b in range(2):
                nc.tensor.matmul(e0f_ps[:, b:b + 1], wexp[:], u0[:, b:b + 1],
                                 start=True, stop=True)
            # h2-independent first MLP matmul, pre-run while attention finishes
            y0_ps = ps.tile([H, SHARD], f32, tag="spB", padded_shape=[128, 512], name="y0_ps")
            nc.tensor.matmul(y0_ps[:], mlpw[0:65, MW_WD0A:MW_WD0A + H], mlpin[:],
                             start=True, stop=True)
            e_b[1] = ps.tile([128, NA], f32, tag="gl", bufs=2, name="eb1")
            for lo, hi in ((0, C0), (C0, C1)):
                nc.tensor.matmul(e_b[1][:, lo:hi], wexp[:], t_sb[1][:, lo:hi],
                                 start=True, stop=True)
            pexp1 = sb.tile([128, NA], bf16, tag="pex", bufs=2)
            nc.scalar.activation(pexp1[:], e_b[1][:], AF.Exp, bias=0.0,
                                 accum_out=ssum[:, 1:2])

            # weighted value sums (fused mul + row-accumulate)
            scr = sb.tile([128, NA], bf16, tag="scr")
            nc.vector.scalar_tensor_tensor(
                out=scr[:], in0=pexp0[:], scalar=1.0, in1=gr_b[0][:],
                op0=ALU.mult, op1=ALU.mult, accum_out=att_u[:, 0:1])
            nc.vector.scalar_tensor_tensor(
                out=scr[:], in0=pexp1[:], scalar=1.0, in1=gr_b[1][:],
                op0=ALU.mult, op1=ALU.mult, accum_out=att_u[:, 1:2])
            nc.scalar.activation(p0c[:, 0:1], e0f_ps[:, 0:1], AF.Exp)
            nc.scalar.activation(p0c[:, 1:2], e0f_ps[:, 1:2], AF.Exp)

            # merge j=0 contribution + softmax-normalize, both blocks at once
            den = sb.tile([128, 2], f32, tag="den")
            nc.vector.tensor_tensor(den[:], ssum[:], p0c[:], op=ALU.add)
            rs = sb.tile([128, 2], f32, tag="rs")
            nc.vector.reciprocal(rs[:], den[:])
            pg = sb.tile([128, 2], f32, tag="pg")
            nc.vector.scalar_tensor_tensor(out=pg[:], in0=p0c[:], scalar=1.0,
                                           in1=gr0c[:], op0=ALU.mult, op1=ALU.mult)
            num = sb.tile([128, 2], f32, tag="num")
            nc.vector.tensor_tensor(num[:], att_u[:], pg[:], op=ALU.add)
            att_n = sb.tile([128, 2], bf16, tag="attn")
            nc.vector.tensor_tensor(att_n[:], num[:], rs[:], op=ALU.mult)

            # ---- final MLP on this core's 128-node shard ----
            # c0 = Wd0b @ h2 via fused G = 0.25*[Wd0b.T; Wd0b.T] (same both blocks)
            c0_ps = ps.tile([H, 2], f32, tag="spA", padded_shape=[128, 512], name="c0_ps")
            nc.tensor.matmul(c0_ps[:], mlpw[0:128, MW_G:MW_G + H], att_n[:],
                             start=True, stop=True)
            c0col = sb.tile([H, 1], f32, tag="c0")
            nc.vector.reduce_sum(c0col[:], c0_ps[:], axis=AX.X)
            y0u = sb.tile([H, SHARD], bf16, tag="y0u")
            nc.vector.tensor_scalar_add(y0u[:], y0_ps[:], c0col[:])
            leaky(y0a[0:64, :], y0u[:])
            y1_ps = ps.tile([128, SHARD], f32, tag="spA", padded_shape=[128, 512], name="y1_ps")
            nc.tensor.matmul(y1_ps[:], mlpw[0:65, MW_WD1:MW_WD1 + 128], y0a[:],
                             start=True, stop=True)
            y1u = sb.tile([128, SHARD], bf16, tag="y1u")
            nc.vector.tensor_copy(y1u[:], y1_ps[:])
            y1 = sb.tile([128, SHARD], bf16, tag="y1")
            leaky(y1[:], y1u[:])
            o_ps = ps.tile([RT, SHARD], f32, tag="spA", padded_shape=[128, 512], name="o_ps")
            nc.tensor.matmul(o_ps[:], mlpw[0:128, MW_WD2:MW_WD2 + RT], y1[:],
                             start=True, stop=True)
            # sigmoid(z) = 0.5 + 0.5*tanh(0.5*z), tanh is in the Exp table set
            th = sb.tile([RT, SHARD], f32, tag="th")
            nc.scalar.activation(th[:], o_ps[:], AF.Tanh, bias=bd2h[:], scale=0.5)
            o_sb = sb.tile([RT, SHARD], f32, tag="o")
            nc.vector.tensor_scalar(o_sb[:], th[:], 0.5, 0.5, ALU.mult, ALU.add)
            nc.sync.dma_start(outT_d[:], o_sb[:])

    nc.compile()
    return nc


def _prep_inputs(inputs):
    import ml_dtypes
    f32 = np.float32
    bf16 = ml_dtypes.bfloat16

    def bf(a):
        return np.ascontiguousarray(np.asarray(a, f32), dtype=f32).astype(bf16)

    hidden = np.asarray(inputs["hidden"], f32)
    ambiguous = np.asarray(inputs["ambiguous"], f32)
    type_agents = np.asarray(inputs["type_agents"], f32)
    W_self = np.asarray(inputs["W_self"], f32)
    b_self = np.asarray(inputs["b_self"], f32)
    W_merge = np.asarray(inputs["W_merge"], f32)
    b_merge = np.asarray(inputs["b_merge"], f32)
    W_trans = np.asarray(inputs["W_trans"], f32)
    b_trans = np.asarray(inputs["b_trans"], f32)
    W_l = np.asarray(inputs["W_l"], f32)
    W_r = np.asarray(inputs["W_r"], f32)
    w_attn = np.asarray(inputs["w_attn"], f32)
    Wd0 = np.asarray(inputs["Wd0"], f32)
    bd0 = np.asarray(inputs["bd0"], f32)
    Wd1 = np.asarray(inputs["Wd1"], f32)
    bd1 = np.asarray(inputs["bd1"], f32)
    Wd2 = np.asarray(inputs["Wd2"], f32)
    bd2 = np.asarray(inputs["bd2"], f32)

    # pro pack [65, 461]
    pro = np.zeros((65, PRO_COLS), f32)
    pro[0:64, PRO_WSELF:PRO_WSELF + H] = W_self.T
    pro[64, PRO_WSELF:PRO_WSELF + H] = b_self
    for t in range(RT):
        pro[0:64, PRO_WT + H * t:PRO_WT + H * (t + 1)] = W_trans[t].T / APT
        pro[64, PRO_WT + H * t:PRO_WT + H * (t + 1)] = b_trans[t]
    pro[0:64, PRO_WMR:PRO_WMR + H] = W_merge[:, H:].T
    pro[64, PRO_WMR:PRO_WMR + H] = b_merge
    pro[0:64, PRO_WML:PRO_WML + H] = W_merge[:, :H].T
    pro[0:64, PRO_HID] = hidden[0]
    pro[64, PRO_HID] = 1.0
    pro[0:64, PRO_TA:PRO_TA + RT * APT] = type_agents.reshape(RT * APT, H).T

    # gatw pack [64, 1535]
    gatw = np.zeros((64, GW_COLS), f32)
    gatw[:, GW_AMB:GW_AMB + N_AMB] = ambiguous.T
    gatw[:, GW_WL:GW_WL + 256] = W_l.T
    gatw[:, GW_WR:GW_WR + 256] = W_r.T

    # block-diagonal w_attn (per-head logit replicated across 64 rows)
    wexp = np.zeros((128, 128), f32)
    for hh in range(2):
        wexp[hh * 64:(hh + 1) * 64, hh * 64:(hh + 1) * 64] = w_attn[:, None]

    # mlpw pack [128, 260]
    mlpw = np.zeros((128, MW_COLS), f32)
    G = 0.25 * Wd0[:, H:].T  # fold (mean over 4 heads) fused into Wd0b
    mlpw[0:64, MW_G:MW_G + H] = G
    mlpw[64:128, MW_G:MW_G + H] = G
    mlpw[0:64, MW_WD1:MW_WD1 + 128] = Wd1.T
    mlpw[64, MW_WD1:MW_WD1 + 128] = bd1
    mlpw[0:64, MW_WD0A:MW_WD0A + H] = Wd0[:, :H].T
    mlpw[64, MW_WD0A:MW_WD0A + H] = bd0
    mlpw[0:128, MW_WD2:MW_WD2 + RT] = Wd2.T

    shared = {
        "pro": bf(pro),
        "gatw": bf(gatw),
        "wexp": bf(wexp),
        "mlpw": bf(mlpw),
        "bd2h": np.ascontiguousarray(0.5 * bd2.reshape(RT, 1), f32),
    }
    amb_pad = np.zeros((65, NCORES * SHARD), f32)
    amb_pad[0:64, :N_AMB] = ambiguous.T
    amb_pad[64, :] = 1.0
    in_maps = []
    for cidx in range(NCORES):
        m = dict(shared)
        m["mlpin"] = bf(amb_pad[:, cidx * SHARD:(cidx + 1) * SHARD])
        in_maps.append(m)
    return in_maps


def kernel(**inputs) -> np.ndarray:
    global _compiled
    if _compiled is None:
        _compiled = _build()
    nc = _compiled
    from concourse import bass_utils

    in_maps = _prep_inputs(inputs)
    res = bass_utils.run_bass_kernel_spmd(nc, in_maps, core_ids=list(range(NCORES)))
    out = np.empty((N_AMB, RT), np.float32)
    for cidx in range(NCORES):
        lo = cidx * SHARD
        hi = min(lo + SHARD, N_AMB)
        out[lo:hi, :] = res.results[cidx]["outT"][:, :hi - lo].T
    return out
